# revision 5
# baseline (speedup 1.0000x reference)
"""DotInteraction Trainium2 kernel.

Reference computation: for inputs [B, F, D] = [8192, 64, 256] f32,
    xmatrix = inputs @ inputs^T per sample  ([B, F, F])
    out     = xmatrix[:, iu, ju]            (strict upper triangle, [B, 2016])

Strategy (pure data parallel over 8 NeuronCores, 1024 samples each):
  * Host pre-transposes each core's slice to X^T layout [kb, d, c, q, h, f]
    (kb = 2 k-blocks of 128 over D; c = 8 chunks of 128 samples;
    sample = c*128 + q*2 + h) and casts to fp16 (PE runs fp16 at 1 col/cycle
    vs fp32's 4, and it halves the HBM-in bytes; accumulation stays fp32).
  * Per pair of samples the stationary operand is [K=128, M=128] (two
    samples' X^T side by side -> full 128-col weight load, FWL-eligible),
    the moving operand is the same AP.  out[128, 128] has the two useful
    Gram blocks on the diagonal quadrants; the off-diagonal cross-sample
    quadrants are never read.
  * One PSUM tile (= one padded bank) per pair, two matmuls accumulating
    over the two k-blocks into the same region — the only PSUM structure
    that keeps every Matmult at <=1 sync-wait (walrus rejects more).
  * A PE nop per chunk absorbs the input-DMA waits; a DVE nop absorbs the
    output-DMA waits of the recycled gram slot.
  * DVE copies each pair's PSUM to SBUF with an fp32->fp16 cast; the full
    Gram tiles are DMAed out in a [half, f, c, q, g] layout so every HBM
    write is an 8KB-contiguous run per partition.
  * Host gathers the strict upper triangle (fixed fancy index) and casts
    to f32.
"""

import os
import sys

import numpy as np

for _p in ("/opt/trn_rl_repo", "/root/.axon_site/_ro/trn_rl_repo"):
    if os.path.isdir(_p) and _p not in sys.path:
        sys.path.insert(0, _p)

import bass_rust  # noqa: E402
from concourse import bacc, bass, mybir, tile  # noqa: E402
from concourse.bass_utils import run_bass_kernel_spmd  # noqa: E402

B, F, D = 8192, 64, 256
N_CORES = 8
B_CORE = B // N_CORES            # 1024
N_CHUNKS = 8                     # chunks per core
CS = B_CORE // N_CHUNKS          # 128 samples per chunk
N_PAIRS = CS // 2                # 64 pairs per chunk
KB = 2                           # k-blocks of 128 over D

FP16 = mybir.dt.float16
FP32 = mybir.dt.float32

_cache = {}


def _dep(a, b, sync, reason):
    bass_rust.add_dep_helper(a.ins, b.ins, sync=sync, reason=reason)


def _build():
    nc = bacc.Bacc()
    # [kb, d, chunk, pair, half, f]
    xt = nc.declare_dram_parameter(
        "xt", [KB, 128, N_CHUNKS, N_PAIRS, 2, F], FP16, isOutput=False
    )
    # [half, f, chunk, pair, g]
    out = nc.declare_dram_parameter(
        "out", [2, F, N_CHUNKS, N_PAIRS, F], FP16, isOutput=True
    )

    with tile.TileContext(nc) as tc:
        with (
            tc.tile_pool(name="x", bufs=6) as xpool,
            tc.tile_pool(name="gram", bufs=3) as gpool,
            tc.tile_pool(name="ps", bufs=8, space=bass.MemorySpace.PSUM) as pspool,
        ):
            prev_last_mm = None
            prev_last_copy = None
            prev_out_dmas = []
            for c in range(N_CHUNKS):
                xk = []
                in_dmas = []
                for kb in range(KB):
                    xtile = xpool.tile([128, N_PAIRS, 2, F], FP16, tag="x")
                    in_dmas.append(
                        nc.sync.dma_start(out=xtile[:], in_=xt[kb, :, c, :, :, :])
                    )
                    xk.append(xtile)

                # PE nops per chunk soak up the input-DMA waits so no
                # Matmult needs more than a single sync-wait.  The NoOp
                # struct itself also only carries one wait, hence one nop
                # per DMA, chained.
                pe_nop = None
                for i, d in enumerate(in_dmas):
                    nop = nc.tensor.nop(hint=f"dma_absorb_c{c}_{i}")
                    _dep(nop, d, True, "absorb input dma wait")
                    if pe_nop is not None:
                        _dep(nop, pe_nop, False, "chain absorb nops")
                    elif prev_last_mm is not None:
                        _dep(nop, prev_last_mm, False, "keep PE stream in order")
                    pe_nop = nop

                # Same trick on DVE for the output DMAs of the chunk whose
                # gram slot is being recycled.
                dve_nop = None
                for i, d in enumerate(prev_out_dmas):
                    nop = nc.vector.nop(hint=f"odma_absorb_c{c}_{i}")
                    _dep(nop, d, True, "absorb output dma wait")
                    if dve_nop is not None:
                        _dep(nop, dve_nop, False, "chain absorb nops")
                    elif prev_last_copy is not None:
                        _dep(nop, prev_last_copy, False, "keep DVE stream in order")
                    dve_nop = nop
                if dve_nop is None:
                    dve_nop = nc.vector.nop(hint=f"odma_absorb_c{c}_0")
                    if prev_last_copy is not None:
                        _dep(dve_nop, prev_last_copy, False, "keep DVE stream order")

                gram = gpool.tile([128, N_PAIRS, 2, F], FP16, tag="gram")

                for q in range(N_PAIRS):
                    ps = pspool.tile([128, 2, F], FP32, tag="ps")
                    s0 = xk[0][:, q, :, :]   # [128, 2, 64]
                    s1 = xk[1][:, q, :, :]
                    mm0 = nc.tensor.matmul(ps[:], s0, s0, start=True, stop=False)
                    _dep(mm0, pe_nop, False, "mm after chunk dma nop")
                    mm1 = nc.tensor.matmul(ps[:], s1, s1, start=False, stop=True)
                    prev_last_mm = mm1

                    cp = nc.vector.tensor_copy(gram[:, q, :, :], ps[:])
                    _dep(cp, dve_nop, False, "copy after chunk odma nop")
                    prev_last_copy = cp

                # sample 2q   lives at partitions 0:64,   columns (q, 0, :)
                # sample 2q+1 lives at partitions 64:128, columns (q, 1, :)
                prev_out_dmas = [
                    nc.sync.dma_start(
                        out=out[0, :, c, :, :], in_=gram[0:64, :, 0, :]
                    ),
                    nc.sync.dma_start(
                        out=out[1, :, c, :, :], in_=gram[64:128, :, 1, :]
                    ),
                ]
    nc.compile()
    return nc


def _get_nc():
    if "nc" not in _cache:
        _cache["nc"] = _build()
    return _cache["nc"]


def kernel(inputs: np.ndarray) -> np.ndarray:
    inputs = np.asarray(inputs)
    assert inputs.shape == (B, F, D), inputs.shape

    in_maps = []
    for core in range(N_CORES):
        xc = inputs[core * B_CORE : (core + 1) * B_CORE]
        # [c, q, h, f, kb, d] -> [kb, d, c, q, h, f]
        xt = (
            xc.reshape(N_CHUNKS, N_PAIRS, 2, F, KB, 128)
            .transpose(4, 5, 0, 1, 2, 3)
            .astype(np.float16)
        )
        in_maps.append({"xt": np.ascontiguousarray(xt)})

    nc = _get_nc()
    res = run_bass_kernel_spmd(nc, in_maps, list(range(N_CORES)))

    iu, ju = np.triu_indices(F, k=1)
    outs = []
    for core in range(N_CORES):
        r = res.results[core]["out"]  # [2, F, c, q, g] fp16
        gram = (
            r.transpose(2, 3, 0, 1, 4)  # [c, q, h, f, g]
            .reshape(B_CORE, F, F)
        )
        outs.append(gram[:, iu, ju])
    return np.concatenate(outs, axis=0).astype(np.float32)
